# revision 25
# baseline (speedup 1.0000x reference)
"""Bahdanau-attention pooling kernel for TRN2, data-parallel over 8 NeuronCores.

Reference computation (per batch b):
    h   = tanh(enc @ W1enc.T + hid @ W1hid.T + b1)    [S, K]   (K = D = 512)
    e   = h @ w2                                       [S]
    a   = softmax(e)                                   [S]
    ctx = a @ enc                                      [D]

Distribution: batch dim (32) sharded 4-per-core across 8 cores; replicated
weights, no collectives.

v4 design (per core, single pass over the encoder stream):
  - scores: enc ships as e4m3 [d, s] pair-tiles; W1enc pre-scaled x16 and
    quantized e4m3; h-matmuls run in DoubleRow fp8 mode (256-deep
    contraction per pass); the 1/16 rescale is folded into the tanh scale.
    tanh is j-pair fused (one ACT per kc chunk covers both tiles of a pair).
  - r = W1hid @ hid + b1 is computed on the HOST (tiny) and shipped as a
    [128, KC, b] column table -- no w1ht/hid/b1 on device.
  - e-matmuls (prev pair) run as FOUR concurrent 1-row chains in distinct
    PE column groups (q0/q32 = kc 0-1 halves, q64/q96 = kc 2-3 halves),
    packed into the kc0/kc1 h-windows; DVE adds the two halves. This halves
    the PE wall-time of the e stage and finishes it early enough that the
    exp -> pT chain never stalls the ctx matmuls.
  - exp runs with accum_out: z falls out of the activation for free (per
    batch it lands in a [33, n_pairs] slot table; rows 0/32 are j=0/1).
    Softmax normalization happens on the host: the kernel returns raw
    4-row ctx partials and the z table.
  - context: runs on the PE. enc also ships as e3m4 [s, d] pair-tiles
    (error-diffusion rounded along s so quantization noise cancels in the
    softmax average), with s interleaved as s = 4p + c so a plain
    [1,512]->[128,4] DMA produces the p-column tiles. Each tile adds 4
    rank-1 matmuls column-tiled to PSUM partitions {0,32,64,96} -- the four
    run fully concurrently on distinct col groups -- accumulated across the
    whole batch in one PSUM bank.
  - pT scatter DMAs ride the (otherwise idle) GpSimd queue; enc streaming
    rides Sync; the Scalar engine does activations only.
"""

import numpy as np

B, S, D = 32, 4096, 512
N_CORES = 8
B_LOC = B // N_CORES
T = 512          # s-tile size
KC = D // 128    # 4 k-chunks
DC = D // 128    # 4 d-chunks
W_SCALE = 16.0   # host pre-scale on W1enc before e4m3 quantization


def build_nc(b_loc=B_LOC, s_len=S, t=T):
    import concourse.bass as bass
    import concourse.mybir as mybir
    import concourse.tile as tile

    fp32 = mybir.dt.float32
    bf16 = mybir.dt.bfloat16
    f8e4 = mybir.dt.float8e4
    f8e3 = mybir.dt.float8e3
    AF = mybir.ActivationFunctionType
    Alu = mybir.AluOpType
    DR = mybir.MatmulPerfMode.DoubleRow

    nc = bass.Bass()

    n_tiles = s_len // t
    n_pairs_b = n_tiles // 2

    enc8_ext = nc.declare_dram_parameter(
        "enc8", [b_loc, n_pairs_b, 128, 2, DC, t], f8e4, isOutput=False)
    encq3_ext = nc.declare_dram_parameter(
        "encq3", [b_loc, n_pairs_b, 128, 2, 4, D], f8e3, isOutput=False)
    w1et8_ext = nc.declare_dram_parameter(
        "w1et8", [KC, 128, DC, 128], f8e4, isOutput=False)
    w28_ext = nc.declare_dram_parameter("w28", [KC, 128], bf16, isOutput=False)
    r_ext = nc.declare_dram_parameter("r", [b_loc, D], fp32, isOutput=False)
    ctx4_ext = nc.declare_dram_parameter("ctx4", [b_loc, 4, D], fp32, isOutput=True)
    z_ext = nc.declare_dram_parameter(
        "z", [b_loc, 2, n_pairs_b], fp32, isOutput=True)

    with tile.TileContext(nc) as tc:
        with (
            tc.tile_pool(name="singles", bufs=1) as singles,
            tc.tile_pool(name="enc8_pool", bufs=3) as enc8_pool,
            tc.tile_pool(name="enc3_pool", bufs=6) as enc3_pool,
            tc.tile_pool(name="h8_pool", bufs=3) as h8_pool,
            tc.tile_pool(name="p_pool", bufs=4) as p_pool,
            tc.tile_pool(name="pt_pool", bufs=5) as pt_pool,
            tc.tile_pool(name="esb_pool", bufs=4) as esb_pool,
            tc.tile_pool(name="ctxsb_pool", bufs=2) as ctxsb_pool,
            tc.tile_pool(name="z_pool", bufs=2) as z_pool,
            tc.tile_pool(name="ps_h", bufs=3, space=bass.MemorySpace.PSUM) as ps_h,
            tc.tile_pool(name="ps_e", bufs=1, space=bass.MemorySpace.PSUM) as ps_e,
            tc.tile_pool(name="ps_c", bufs=1, space=bass.MemorySpace.PSUM) as ps_c,
        ):
            # ---- persistent tiles; ordered so pair-0's dependencies land
            # first: w1et8[kc0] -> (enc8 pair 0 goes right after, see loop)
            w1et8_sb = singles.tile([128, KC, DC, 128], f8e4)
            w2_col = singles.tile([128, KC], bf16)
            r_sb = singles.tile([128, b_loc, KC], fp32)   # [p(k), b, k-chunk]
            nc.sync.dma_start(out=w1et8_sb[:, 0, :, :], in_=w1et8_ext[0])

            ctx_ps = ps_c.tile([128, t], fp32, tag="ctx")

            # ---- main loop: flat software-pipelined stream of tile pairs ---
            # Per pair P (b, it0) we emit, in order:
            #   DMAs(P) + deferred enc3 DMA(P-1)
            #   kc-loop: h-matmuls(P); at kc 0/1 the e-matmuls(P-1) run as 4
            #            concurrent col-group chains (kc-halves x j)
            #   DVE half-add + exp/z/pT(P-1)  (one ACT op, one transpose DMA)
            #   ctx-matmuls(P-2) (their pT landed during P-1 -- never stalls)
            # The pipeline runs across batch boundaries; each batch epilogue
            # is emitted right after that batch's last ctx-matmul flush.
            pairs = [(b, it0) for b in range(b_loc) for it0 in range(0, n_tiles, 2)]
            prev = None          # (h8, enc3t, b, it0) of pair P-1
            ctx_q = []           # pending ctx items: (pT, j, e3t, b, ti)
            zparts = {}          # per-batch z slot tables

            def emit_ctx(item):
                pT, j, e3t, cb, ti = item
                for c in range(4):
                    nc.tensor.matmul(
                        ctx_ps[32 * c:32 * c + 1, :],
                        pT[:, j, c:c + 1],
                        e3t[:, j, c, :],
                        start=(ti == 0),
                        stop=(ti == n_tiles - 1),
                        tile_position=(0, 32 * c),
                        skip_group_check=True,
                    )

            def emit_epilogue(eb):
                # raw 4-row ctx partials + z slots out; host sums/normalizes.
                # These ride Sync (not GpSimd) so the latency-critical pT
                # scatters never queue behind them.
                ctx_sb = ctxsb_pool.tile([97, t], fp32, tag="ctxsb")
                nc.vector.tensor_copy(out=ctx_sb, in_=ctx_ps[0:97, :])
                nc.sync.dma_start(
                    out=ctx4_ext[eb], in_=ctx_sb[0:97:32, :])
                zp = zparts.pop(eb)
                nc.sync.dma_start(out=z_ext[eb, 0], in_=zp[0:1, :])
                nc.sync.dma_start(out=z_ext[eb, 1], in_=zp[32:33, :])

            def emit_exp_block(pprev, split=False):
                # e halves add (DVE), exp (ACT), z row-sum (DVE), pT scatter
                # (GpSimd queue): all for pair P-1.  split=True runs exp per
                # j-row so the first pT scatter starts earlier (drain only).
                ph8, penc3, pb, pit0 = pprev
                ppi = pit0 // 2
                # DVE cannot read two PSUM operands in one op: stage the
                # upper half in SBUF first
                e_hi = esb_pool.tile([33, t], fp32, tag="ehi", name="e_hi")
                nc.vector.tensor_copy(out=e_hi, in_=e_ps[64:97, :])
                e_sb = esb_pool.tile([33, t], fp32, tag="esb")
                nc.vector.tensor_add(
                    out=e_sb, in0=e_ps[0:33, :], in1=e_hi)
                if ppi == 0:
                    zparts[pb] = z_pool.tile(
                        [33, n_pairs_b], fp32, tag="zp", name=f"zp{pb}")
                p_sb = p_pool.tile([33, t], bf16, tag="p")
                pT = pt_pool.tile([128, 2, 4], bf16, tag="pt")
                if split:
                    for j in range(2):
                        nc.scalar.activation(
                            out=p_sb[32 * j:32 * j + 1, :],
                            in_=e_sb[32 * j:32 * j + 1, :], func=AF.Exp)
                        nc.gpsimd.dma_start(
                            out=pT[:, j, :], in_=p_sb[32 * j:32 * j + 1, :])
                        nc.vector.tensor_reduce(
                            out=zparts[pb][32 * j:32 * j + 1, ppi:ppi + 1],
                            in_=p_sb[32 * j:32 * j + 1, :],
                            axis=mybir.AxisListType.X, op=Alu.add)
                else:
                    nc.scalar.activation(out=p_sb, in_=e_sb, func=AF.Exp)
                    for j in range(2):
                        nc.gpsimd.dma_start(
                            out=pT[:, j, :], in_=p_sb[32 * j:32 * j + 1, :])
                    nc.vector.tensor_reduce(
                        out=zparts[pb][:, ppi:ppi + 1], in_=p_sb,
                        axis=mybir.AxisListType.X, op=Alu.add)
                for j in range(2):
                    ctx_q.append((pT, j, penc3, pb, pit0 + j))

            pending_e3 = []
            pending_epi = []
            for pi, (b, it0) in enumerate(pairs):
                e8 = enc8_pool.tile([128, 2, DC, t], f8e4, tag="enc8")
                if pi == 0:
                    # split pair-0's load so the kc loop can start after the
                    # first 256 KB
                    nc.sync.dma_start(out=e8[:, 0], in_=enc8_ext[b, 0, :, 0])
                    nc.sync.dma_start(out=e8[:, 1], in_=enc8_ext[b, 0, :, 1])
                else:
                    nc.sync.dma_start(out=e8, in_=enc8_ext[b, it0 // 2])
                e3 = enc3_pool.tile([128, 2, 4, D], f8e3, tag="enc3")
                pending_e3.append((e3, b, it0 // 2))
                if pi == 0:
                    # rest of the persistent loads ride the GpSimd ring (idle
                    # until pair 1's pT) so sync's queue holds only the
                    # critical enc stream
                    nc.gpsimd.dma_start(
                        out=r_sb, in_=r_ext.rearrange("b (c p) -> p b c", p=128))
                    nc.gpsimd.dma_start(out=w1et8_sb[:, 1, :, :], in_=w1et8_ext[1])
                    nc.gpsimd.dma_start(
                        out=w2_col, in_=w28_ext.rearrange("c p -> p c"))
                    for kc in range(2, KC):
                        nc.gpsimd.dma_start(
                            out=w1et8_sb[:, kc, :, :], in_=w1et8_ext[kc])
                while len(pending_e3) > 2:
                    pe3, pe3b, pe3pi = pending_e3.pop(0)
                    nc.sync.dma_start(out=pe3, in_=encq3_ext[pe3b, pe3pi])

                e_ps = ps_e.tile([128, t], fp32, tag="e", name="eps") if prev is not None else None

                # h8[p(k), kc, j, s] = tanh((1/16) h_ps + r)
                h8 = h8_pool.tile([128, KC, 2, t], bf16, tag="h8")
                for kc in range(KC):
                    h_ps = ps_h.tile([128, 2, t], fp32, tag="h")
                    for c2 in range(2):
                        for j in range(2):
                            nc.tensor.matmul(
                                h_ps[:, j, :],
                                w1et8_sb[:, kc, 2 * c2:2 * c2 + 2, :],
                                e8[:, j, 2 * c2:2 * c2 + 2, :],
                                start=(c2 == 0),
                                stop=(c2 == 1),
                                perf_mode=DR,
                                skip_group_check=True,
                            )
                    if prev is not None and kc < 2:
                        # e-matmuls of the previous pair: four 1-row chains
                        # (kc-half x j) on distinct PE column groups run
                        # concurrently; each chain is 2 steps (kc, kc+2)
                        for j in range(2):
                            nc.tensor.matmul(
                                e_ps[32 * j:32 * j + 1, :],
                                w2_col[:, kc:kc + 1],
                                prev[0][:, kc, j, :],
                                start=(kc == 0),
                                stop=(kc == 1),
                                tile_position=(0, 32 * j),
                                skip_group_check=True,
                            )
                            nc.tensor.matmul(
                                e_ps[64 + 32 * j:65 + 32 * j, :],
                                w2_col[:, kc + 2:kc + 3],
                                prev[0][:, kc + 2, j, :],
                                start=(kc == 0),
                                stop=(kc == 1),
                                tile_position=(0, 64 + 32 * j),
                                skip_group_check=True,
                            )
                    nc.scalar.activation(
                        out=h8[:, kc, :, :], in_=h_ps, func=AF.Tanh,
                        bias=r_sb[:, b, kc:kc + 1], scale=1.0 / W_SCALE,
                    )
                    if prev is not None and kc == 2:
                        emit_exp_block(prev)

                # batch epilogues are delayed one pair past their last ctx
                # quad so the PSUM->SBUF copy never head-of-line-blocks the
                # DVE FIFO (the e-half add behind it feeds exp -> pT -> ctx)
                while pending_epi:
                    emit_epilogue(pending_epi.pop(0))
                # flush ctx items four pairs old: the pT scatter DMA takes
                # ~3us on the small-descriptor queue (worse while the initial
                # enc prefetch burst is in flight), so keep generous slack
                while len(ctx_q) > 6:
                    item = ctx_q.pop(0)
                    emit_ctx(item)
                    if item[4] == n_tiles - 1:
                        pending_epi.append(item[3])

                prev = (h8, e3, b, it0)

            # drain: e/exp/pT of the final pair, remaining ctx, last epilogue
            while pending_e3:
                pe3, pe3b, pe3pi = pending_e3.pop(0)
                nc.sync.dma_start(out=pe3, in_=encq3_ext[pe3b, pe3pi])
            # two stale ctx items (pair P-1) can flush before the final
            # e-chains -- their pT is already resident
            while pending_epi:
                emit_epilogue(pending_epi.pop(0))
            while len(ctx_q) > 2:
                item = ctx_q.pop(0)
                emit_ctx(item)
                if item[4] == n_tiles - 1:
                    pending_epi.append(item[3])
            # final pair uses plain 2-way chains (all four kc steps into rows
            # 0/32): no half-add needed, so exp can read PSUM directly -- the
            # drain's serial latency shrinks by the DVE copy+add
            e_ps = ps_e.tile([128, t], fp32, tag="e", name="eps_last")
            for kc in range(KC):
                for j in range(2):
                    nc.tensor.matmul(
                        e_ps[32 * j:32 * j + 1, :],
                        w2_col[:, kc:kc + 1],
                        prev[0][:, kc, j, :],
                        start=(kc == 0),
                        stop=(kc == KC - 1),
                        tile_position=(0, 32 * j),
                        skip_group_check=True,
                    )
            ph8, penc3, pb, pit0 = prev
            ppi = pit0 // 2
            p_sb = p_pool.tile([33, t], bf16, tag="p", name="p_last")
            pT = pt_pool.tile([128, 2, 4], bf16, tag="pt", name="pt_last")
            for j in range(2):
                nc.scalar.activation(
                    out=p_sb[32 * j:32 * j + 1, :],
                    in_=e_ps[32 * j:32 * j + 1, :], func=AF.Exp)
                nc.gpsimd.dma_start(
                    out=pT[:, j, :], in_=p_sb[32 * j:32 * j + 1, :])
                nc.vector.tensor_reduce(
                    out=zparts[pb][32 * j:32 * j + 1, ppi:ppi + 1],
                    in_=p_sb[32 * j:32 * j + 1, :],
                    axis=mybir.AxisListType.X, op=Alu.add)
            for j in range(2):
                ctx_q.append((pT, j, penc3, pb, pit0 + j))
            while pending_epi:
                emit_epilogue(pending_epi.pop(0))
            while ctx_q:
                item = ctx_q.pop(0)
                emit_ctx(item)
                if item[4] == n_tiles - 1:
                    emit_epilogue(item[3])

    return nc


# Instruction opcodes whose ISA structs tolerate multi-waits (or that the
# split must not touch). Everything else on this walrus build has a single
# sync-wait slot, so excess waits move onto preceding same-engine NoOps.
_NO_SPLIT = {"EventSemaphore", "Call", "UnconditionalBranch", "RegisterMove"}


def split_multi_waits(nc, limit=1):
    import concourse.mybir as mybir

    ctr = 0
    for fn in nc.m.functions:
        for blk in fn.blocks:
            new = []
            for inst in blk.instructions:
                si = inst.sync_info
                waits = list(si.on_wait) if si is not None and si.on_wait else []
                if inst.opcode not in _NO_SPLIT and len(waits) > limit:
                    extra, keep = waits[:-limit], waits[-limit:]
                    for w in extra:
                        ctr += 1
                        new.append(mybir.InstNoOp(
                            name=f"WSPLIT-{ctr}", engine=inst.engine,
                            sync_info=mybir.SyncInfo(on_wait=[w], on_update=[])))
                    inst.sync_info = mybir.SyncInfo(
                        on_wait=keep,
                        on_update=list(si.on_update) if si.on_update else [])
                new.append(inst)
            blk.instructions = new
    return ctr


def _diffuse_quant(x, qdtype):
    """Error-diffusion rounding of x (f32) to qdtype along the last axis:
    running quantization error is fed into the next element, so partial sums
    of the quantized stream track the exact partial sums within half an ULP.
    """
    out = np.empty(x.shape, dtype=qdtype)
    c = np.zeros(x.shape[:-1], dtype=np.float32)
    for s in range(x.shape[-1]):
        v = x[..., s] + c
        q = v.astype(qdtype)
        out[..., s] = q
        c = v - q.astype(np.float32)
    return out


def _prep_host(hidden_state, encoder_output, W1, b1, w2):
    import ml_dtypes

    bf16 = ml_dtypes.bfloat16
    f8e4 = ml_dtypes.float8_e4m3
    f8e3 = ml_dtypes.float8_e3m4

    n_tiles = S // T
    n_pairs = n_tiles // 2
    encT = encoder_output.transpose(0, 2, 1)                 # [B, D, S] f32
    # score copy: e4m3, [b, pr, p, j, dc, s'] with d = dc*128 + p,
    # s = (2 pr + j) T + s'
    enc8 = np.ascontiguousarray(
        encT.reshape(B, DC, 128, n_pairs, 2, T)
        .transpose(0, 3, 2, 4, 1, 5).astype(f8e4)
    )
    # context copy: e3m4 diffused along s, [b, pr, p, j, c, d] with
    # s = (2 pr + j) T + 4 p + c
    encq = _diffuse_quant(encT, f8e3).astype(f8e3)           # [B, D, S]
    encq3 = np.ascontiguousarray(
        encq.transpose(0, 2, 1).reshape(B, n_pairs, 2, 128, 4, D)
        .transpose(0, 1, 3, 2, 4, 5)
    )
    w1eT = (W_SCALE * W1[:, :D].T).astype(f8e4)              # [d, k]
    w1et8 = np.ascontiguousarray(
        w1eT.reshape(DC, 128, KC, 128).transpose(2, 1, 0, 3)
    )
    w28 = np.ascontiguousarray(w2.reshape(KC, 128).astype(bf16))
    # r = W1hid @ hid + b1 on the host (tiny): [B, K]
    r_all = hidden_state @ W1[:, D:].T + b1
    in_maps = []
    for i in range(N_CORES):
        sl = slice(i * B_LOC, (i + 1) * B_LOC)
        in_maps.append({
            "enc8": np.ascontiguousarray(enc8[sl]),
            "encq3": np.ascontiguousarray(encq3[sl]),
            "w1et8": w1et8,
            "w28": w28,
            "r": np.ascontiguousarray(r_all[sl].astype(np.float32)),
        })
    return in_maps


def _ensure_ntff_hook():
    """Install the axon NTFF profile hook if the image lacks antenv.axon_hooks."""
    import sys
    import types

    try:
        import antenv.axon_hooks  # noqa: F401
        return
    except ImportError:
        pass
    import antenv

    mod = types.ModuleType("antenv.axon_hooks")
    state = {"hook": None}
    mod.set_axon_ntff_profile_hook = lambda h: state.__setitem__("hook", h)
    mod.get_axon_ntff_profile_hook = lambda: state["hook"]
    sys.modules["antenv.axon_hooks"] = mod
    antenv.axon_hooks = mod
    try:
        from trn_agent_boot.trn_boot import _ntff_profile_via_ctypes

        mod.set_axon_ntff_profile_hook(
            _ntff_profile_via_ctypes("/opt/axon/libaxon_pjrt.so")
        )
    except Exception:
        pass


def run(hidden_state, encoder_output, W1, b1, w2, trace=False):
    from concourse.bass_utils import run_bass_kernel_spmd

    if trace:
        _ensure_ntff_hook()

    nc = build_nc()
    nc.finalize()
    split_multi_waits(nc)
    in_maps = _prep_host(
        np.asarray(hidden_state, dtype=np.float32),
        np.asarray(encoder_output, dtype=np.float32),
        np.asarray(W1, dtype=np.float32),
        np.asarray(b1, dtype=np.float32),
        np.asarray(w2, dtype=np.float32),
    )
    res = run_bass_kernel_spmd(nc, in_maps, core_ids=list(range(N_CORES)), trace=trace)
    outs = []
    for i in range(N_CORES):
        ctx4 = res.results[i]["ctx4"]                    # [b_loc, 4, D]
        z = res.results[i]["z"]                          # [b_loc, 2, n_pairs]
        zb = z.reshape(B_LOC, -1).sum(axis=1)            # [b_loc]
        outs.append(ctx4.sum(axis=1) / zb[:, None])
    out = np.concatenate(outs, axis=0).astype(np.float32)
    return out, res


def kernel(**inputs):
    out, _ = run(**inputs)
    return out


# revision 27
# speedup vs baseline: 1.0126x; 1.0126x over previous
"""Bahdanau-attention pooling kernel for TRN2, data-parallel over 8 NeuronCores.

Reference computation (per batch b):
    h   = tanh(enc @ W1enc.T + hid @ W1hid.T + b1)    [S, K]   (K = D = 512)
    e   = h @ w2                                       [S]
    a   = softmax(e)                                   [S]
    ctx = a @ enc                                      [D]

Distribution: batch dim (32) sharded 4-per-core across 8 cores; replicated
weights, no collectives.

v4 design (per core, single pass over the encoder stream):
  - scores: enc ships as e4m3 [d, s] pair-tiles; W1enc pre-scaled x16 and
    quantized e4m3; h-matmuls run in DoubleRow fp8 mode (256-deep
    contraction per pass); the 1/16 rescale is folded into the tanh scale.
    tanh is j-pair fused (one ACT per kc chunk covers both tiles of a pair).
  - r = W1hid @ hid + b1 is computed on the HOST (tiny) and shipped as a
    [128, KC, b] column table -- no w1ht/hid/b1 on device.
  - e-matmuls (prev pair) run as FOUR concurrent 1-row chains in distinct
    PE column groups (q0/q32 = kc 0-1 halves, q64/q96 = kc 2-3 halves),
    packed into the kc0/kc1 h-windows; DVE adds the two halves. This halves
    the PE wall-time of the e stage and finishes it early enough that the
    exp -> pT chain never stalls the ctx matmuls.
  - exp runs with accum_out: z falls out of the activation for free (per
    batch it lands in a [33, n_pairs] slot table; rows 0/32 are j=0/1).
    Softmax normalization happens on the host: the kernel returns raw
    4-row ctx partials and the z table.
  - context: runs on the PE. enc also ships as e3m4 [s, d] pair-tiles
    (error-diffusion rounded along s so quantization noise cancels in the
    softmax average), with s interleaved as s = 4p + c so a plain
    [1,512]->[128,4] DMA produces the p-column tiles. Each tile adds 4
    rank-1 matmuls column-tiled to PSUM partitions {0,32,64,96} -- the four
    run fully concurrently on distinct col groups -- accumulated across the
    whole batch in one PSUM bank.
  - pT scatter DMAs ride the (otherwise idle) GpSimd queue; enc streaming
    rides Sync; the Scalar engine does activations only.
"""

import numpy as np

B, S, D = 32, 4096, 512
N_CORES = 8
B_LOC = B // N_CORES
T = 512          # s-tile size
KC = D // 128    # 4 k-chunks
DC = D // 128    # 4 d-chunks
W_SCALE = 16.0   # host pre-scale on W1enc before e4m3 quantization


def build_nc(b_loc=B_LOC, s_len=S, t=T):
    import concourse.bass as bass
    import concourse.mybir as mybir
    import concourse.tile as tile

    fp32 = mybir.dt.float32
    bf16 = mybir.dt.bfloat16
    f8e4 = mybir.dt.float8e4
    f8e3 = mybir.dt.float8e3
    AF = mybir.ActivationFunctionType
    Alu = mybir.AluOpType
    DR = mybir.MatmulPerfMode.DoubleRow

    nc = bass.Bass()

    n_tiles = s_len // t
    n_pairs_b = n_tiles // 2

    enc8_ext = nc.declare_dram_parameter(
        "enc8", [b_loc, n_pairs_b, 128, 2, DC, t], f8e4, isOutput=False)
    encq3_ext = nc.declare_dram_parameter(
        "encq3", [b_loc, n_pairs_b, 128, 2, 4, D], f8e3, isOutput=False)
    w1et8_ext = nc.declare_dram_parameter(
        "w1et8", [KC, 128, DC, 128], f8e4, isOutput=False)
    w28_ext = nc.declare_dram_parameter("w28", [KC, 128], bf16, isOutput=False)
    r_ext = nc.declare_dram_parameter("r", [b_loc, D], fp32, isOutput=False)
    ctx4_ext = nc.declare_dram_parameter("ctx4", [b_loc, 4, D], fp32, isOutput=True)
    z_ext = nc.declare_dram_parameter(
        "z", [b_loc, 2, n_pairs_b], fp32, isOutput=True)

    with tile.TileContext(nc) as tc:
        with (
            tc.tile_pool(name="singles", bufs=1) as singles,
            tc.tile_pool(name="enc8_pool", bufs=3) as enc8_pool,
            tc.tile_pool(name="enc3_pool", bufs=6) as enc3_pool,
            tc.tile_pool(name="h8_pool", bufs=3) as h8_pool,
            tc.tile_pool(name="p_pool", bufs=4) as p_pool,
            tc.tile_pool(name="pt_pool", bufs=5) as pt_pool,
            tc.tile_pool(name="esb_pool", bufs=4) as esb_pool,
            tc.tile_pool(name="ctxsb_pool", bufs=2) as ctxsb_pool,
            tc.tile_pool(name="z_pool", bufs=2) as z_pool,
            tc.tile_pool(name="ps_h", bufs=3, space=bass.MemorySpace.PSUM) as ps_h,
            tc.tile_pool(name="ps_e", bufs=1, space=bass.MemorySpace.PSUM) as ps_e,
            tc.tile_pool(name="ps_c", bufs=1, space=bass.MemorySpace.PSUM) as ps_c,
        ):
            # ---- persistent tiles; ordered so pair-0's dependencies land
            # first: w1et8[kc0] -> (enc8 pair 0 goes right after, see loop)
            w1et8_sb = singles.tile([128, KC, DC, 128], f8e4)
            w2_col = singles.tile([128, KC], bf16)
            r_sb = singles.tile([128, b_loc, KC], fp32)   # [p(k), b, k-chunk]
            nc.sync.dma_start(out=w1et8_sb[:, 0, :, :], in_=w1et8_ext[0])

            ctx_ps = ps_c.tile([128, t], fp32, tag="ctx")

            # ---- main loop: flat software-pipelined stream of tile pairs ---
            # Per pair P (b, it0) we emit, in order:
            #   DMAs(P) + deferred enc3 DMA(P-1)
            #   kc-loop: h-matmuls(P); at kc 0/1 the e-matmuls(P-1) run as 4
            #            concurrent col-group chains (kc-halves x j)
            #   DVE half-add + exp/z/pT(P-1)  (one ACT op, one transpose DMA)
            #   ctx-matmuls(P-2) (their pT landed during P-1 -- never stalls)
            # The pipeline runs across batch boundaries; each batch epilogue
            # is emitted right after that batch's last ctx-matmul flush.
            pairs = [(b, it0) for b in range(b_loc) for it0 in range(0, n_tiles, 2)]
            prev = None          # (h8, enc3t, b, it0) of pair P-1
            ctx_q = []           # pending ctx items: (pT, j, e3t, b, ti)
            zparts = {}          # per-batch z slot tables

            def emit_ctx(item):
                pT, j, e3t, cb, ti = item
                for c in range(4):
                    nc.tensor.matmul(
                        ctx_ps[32 * c:32 * c + 1, :],
                        pT[:, j, c:c + 1],
                        e3t[:, j, c, :],
                        start=(ti == 0),
                        stop=(ti == n_tiles - 1),
                        tile_position=(0, 32 * c),
                        skip_group_check=True,
                    )

            def emit_epilogue(eb):
                # raw 4-row ctx partials + z slots out; host sums/normalizes.
                # These ride Sync (not GpSimd) so the latency-critical pT
                # scatters never queue behind them.
                ctx_sb = ctxsb_pool.tile([97, t], fp32, tag="ctxsb")
                nc.vector.tensor_copy(out=ctx_sb, in_=ctx_ps[0:97, :])
                nc.sync.dma_start(
                    out=ctx4_ext[eb], in_=ctx_sb[0:97:32, :])
                zp = zparts.pop(eb)
                nc.sync.dma_start(out=z_ext[eb, 0], in_=zp[0:1, :])
                nc.sync.dma_start(out=z_ext[eb, 1], in_=zp[32:33, :])

            def emit_exp_block(pprev, split=False):
                # e halves add (DVE), exp (ACT), z row-sum (DVE), pT scatter
                # (GpSimd queue): all for pair P-1.  split=True runs exp per
                # j-row so the first pT scatter starts earlier (drain only).
                ph8, penc3, pb, pit0 = pprev
                ppi = pit0 // 2
                # DVE cannot read two PSUM operands in one op: stage the
                # upper half in SBUF first
                e_hi = esb_pool.tile([33, t], fp32, tag="ehi", name="e_hi")
                nc.vector.tensor_copy(out=e_hi, in_=e_ps[64:97, :])
                e_sb = esb_pool.tile([33, t], fp32, tag="esb")
                nc.vector.tensor_add(
                    out=e_sb, in0=e_ps[0:33, :], in1=e_hi)
                if ppi == 0:
                    zparts[pb] = z_pool.tile(
                        [33, n_pairs_b], fp32, tag="zp", name=f"zp{pb}")
                p_sb = p_pool.tile([33, t], bf16, tag="p")
                pT = pt_pool.tile([128, 2, 4], bf16, tag="pt")
                if split:
                    for j in range(2):
                        nc.scalar.activation(
                            out=p_sb[32 * j:32 * j + 1, :],
                            in_=e_sb[32 * j:32 * j + 1, :], func=AF.Exp)
                        nc.gpsimd.dma_start(
                            out=pT[:, j, :], in_=p_sb[32 * j:32 * j + 1, :])
                        nc.vector.tensor_reduce(
                            out=zparts[pb][32 * j:32 * j + 1, ppi:ppi + 1],
                            in_=p_sb[32 * j:32 * j + 1, :],
                            axis=mybir.AxisListType.X, op=Alu.add)
                else:
                    nc.scalar.activation(out=p_sb, in_=e_sb, func=AF.Exp)
                    for j in range(2):
                        nc.gpsimd.dma_start(
                            out=pT[:, j, :], in_=p_sb[32 * j:32 * j + 1, :])
                    nc.vector.tensor_reduce(
                        out=zparts[pb][:, ppi:ppi + 1], in_=p_sb,
                        axis=mybir.AxisListType.X, op=Alu.add)
                for j in range(2):
                    ctx_q.append((pT, j, penc3, pb, pit0 + j))

            pending_e3 = []
            pending_epi = []
            for pi, (b, it0) in enumerate(pairs):
                e8 = enc8_pool.tile([128, 2, DC, t], f8e4, tag="enc8")
                if pi == 0:
                    # split pair-0's load so the kc loop can start after the
                    # first 256 KB
                    nc.sync.dma_start(out=e8[:, 0], in_=enc8_ext[b, 0, :, 0])
                    nc.sync.dma_start(out=e8[:, 1], in_=enc8_ext[b, 0, :, 1])
                else:
                    nc.sync.dma_start(out=e8, in_=enc8_ext[b, it0 // 2])
                e3 = enc3_pool.tile([128, 2, 4, D], f8e3, tag="enc3")
                pending_e3.append((e3, b, it0 // 2))
                if pi == 0:
                    # rest of the persistent loads ride the GpSimd ring (idle
                    # until pair 1's pT) so sync's queue holds only the
                    # critical enc stream
                    nc.scalar.dma_start(
                        out=r_sb, in_=r_ext.rearrange("b (c p) -> p b c", p=128))
                    nc.scalar.dma_start(out=w1et8_sb[:, 1, :, :], in_=w1et8_ext[1])
                    nc.scalar.dma_start(
                        out=w2_col, in_=w28_ext.rearrange("c p -> p c"))
                    for kc in range(2, KC):
                        nc.scalar.dma_start(
                            out=w1et8_sb[:, kc, :, :], in_=w1et8_ext[kc])
                while len(pending_e3) > 2:
                    pe3, pe3b, pe3pi = pending_e3.pop(0)
                    nc.sync.dma_start(out=pe3, in_=encq3_ext[pe3b, pe3pi])

                e_ps = ps_e.tile([128, t], fp32, tag="e", name="eps") if prev is not None else None

                # h8[p(k), kc, j, s] = tanh((1/16) h_ps + r)
                h8 = h8_pool.tile([128, KC, 2, t], bf16, tag="h8")
                for kc in range(KC):
                    h_ps = ps_h.tile([128, 2, t], fp32, tag="h")
                    for c2 in range(2):
                        for j in range(2):
                            nc.tensor.matmul(
                                h_ps[:, j, :],
                                w1et8_sb[:, kc, 2 * c2:2 * c2 + 2, :],
                                e8[:, j, 2 * c2:2 * c2 + 2, :],
                                start=(c2 == 0),
                                stop=(c2 == 1),
                                perf_mode=DR,
                                skip_group_check=True,
                            )
                    if prev is not None and kc < 2:
                        # e-matmuls of the previous pair: four 1-row chains
                        # (kc-half x j) on distinct PE column groups run
                        # concurrently; each chain is 2 steps (kc, kc+2)
                        for j in range(2):
                            nc.tensor.matmul(
                                e_ps[32 * j:32 * j + 1, :],
                                w2_col[:, kc:kc + 1],
                                prev[0][:, kc, j, :],
                                start=(kc == 0),
                                stop=(kc == 1),
                                tile_position=(0, 32 * j),
                                skip_group_check=True,
                            )
                            nc.tensor.matmul(
                                e_ps[64 + 32 * j:65 + 32 * j, :],
                                w2_col[:, kc + 2:kc + 3],
                                prev[0][:, kc + 2, j, :],
                                start=(kc == 0),
                                stop=(kc == 1),
                                tile_position=(0, 64 + 32 * j),
                                skip_group_check=True,
                            )
                    nc.scalar.activation(
                        out=h8[:, kc, :, :], in_=h_ps, func=AF.Tanh,
                        bias=r_sb[:, b, kc:kc + 1], scale=1.0 / W_SCALE,
                    )
                    if prev is not None and kc == 2:
                        emit_exp_block(prev)

                # batch epilogues are delayed one pair past their last ctx
                # quad so the PSUM->SBUF copy never head-of-line-blocks the
                # DVE FIFO (the e-half add behind it feeds exp -> pT -> ctx)
                while pending_epi:
                    emit_epilogue(pending_epi.pop(0))
                # flush ctx items four pairs old: the pT scatter DMA takes
                # ~3us on the small-descriptor queue (worse while the initial
                # enc prefetch burst is in flight), so keep generous slack
                while len(ctx_q) > 6:
                    item = ctx_q.pop(0)
                    emit_ctx(item)
                    if item[4] == n_tiles - 1:
                        pending_epi.append(item[3])

                prev = (h8, e3, b, it0)

            # drain: e/exp/pT of the final pair, remaining ctx, last epilogue
            while pending_e3:
                pe3, pe3b, pe3pi = pending_e3.pop(0)
                nc.sync.dma_start(out=pe3, in_=encq3_ext[pe3b, pe3pi])
            # two stale ctx items (pair P-1) can flush before the final
            # e-chains -- their pT is already resident
            while pending_epi:
                emit_epilogue(pending_epi.pop(0))
            while len(ctx_q) > 2:
                item = ctx_q.pop(0)
                emit_ctx(item)
                if item[4] == n_tiles - 1:
                    pending_epi.append(item[3])
            # final pair uses plain 2-way chains (all four kc steps into rows
            # 0/32): no half-add needed, so exp can read PSUM directly -- the
            # drain's serial latency shrinks by the DVE copy+add
            e_ps = ps_e.tile([128, t], fp32, tag="e", name="eps_last")
            for kc in range(KC):
                for j in range(2):
                    nc.tensor.matmul(
                        e_ps[32 * j:32 * j + 1, :],
                        w2_col[:, kc:kc + 1],
                        prev[0][:, kc, j, :],
                        start=(kc == 0),
                        stop=(kc == KC - 1),
                        tile_position=(0, 32 * j),
                        skip_group_check=True,
                    )
            ph8, penc3, pb, pit0 = prev
            ppi = pit0 // 2
            p_sb = p_pool.tile([33, t], bf16, tag="p", name="p_last")
            pT = pt_pool.tile([128, 2, 4], bf16, tag="pt", name="pt_last")
            for j in range(2):
                nc.scalar.activation(
                    out=p_sb[32 * j:32 * j + 1, :],
                    in_=e_ps[32 * j:32 * j + 1, :], func=AF.Exp)
                nc.gpsimd.dma_start(
                    out=pT[:, j, :], in_=p_sb[32 * j:32 * j + 1, :])
                nc.vector.tensor_reduce(
                    out=zparts[pb][32 * j:32 * j + 1, ppi:ppi + 1],
                    in_=p_sb[32 * j:32 * j + 1, :],
                    axis=mybir.AxisListType.X, op=Alu.add)
            for j in range(2):
                ctx_q.append((pT, j, penc3, pb, pit0 + j))
            while pending_epi:
                emit_epilogue(pending_epi.pop(0))
            while ctx_q:
                item = ctx_q.pop(0)
                emit_ctx(item)
                if item[4] == n_tiles - 1:
                    emit_epilogue(item[3])

    return nc


# Instruction opcodes whose ISA structs tolerate multi-waits (or that the
# split must not touch). Everything else on this walrus build has a single
# sync-wait slot, so excess waits move onto preceding same-engine NoOps.
_NO_SPLIT = {"EventSemaphore", "Call", "UnconditionalBranch", "RegisterMove"}


def split_multi_waits(nc, limit=1):
    import concourse.mybir as mybir

    ctr = 0
    for fn in nc.m.functions:
        for blk in fn.blocks:
            new = []
            for inst in blk.instructions:
                si = inst.sync_info
                waits = list(si.on_wait) if si is not None and si.on_wait else []
                if inst.opcode not in _NO_SPLIT and len(waits) > limit:
                    extra, keep = waits[:-limit], waits[-limit:]
                    for w in extra:
                        ctr += 1
                        new.append(mybir.InstNoOp(
                            name=f"WSPLIT-{ctr}", engine=inst.engine,
                            sync_info=mybir.SyncInfo(on_wait=[w], on_update=[])))
                    inst.sync_info = mybir.SyncInfo(
                        on_wait=keep,
                        on_update=list(si.on_update) if si.on_update else [])
                new.append(inst)
            blk.instructions = new
    return ctr


def _diffuse_quant(x, qdtype):
    """Error-diffusion rounding of x (f32) to qdtype along the last axis:
    running quantization error is fed into the next element, so partial sums
    of the quantized stream track the exact partial sums within half an ULP.
    """
    out = np.empty(x.shape, dtype=qdtype)
    c = np.zeros(x.shape[:-1], dtype=np.float32)
    for s in range(x.shape[-1]):
        v = x[..., s] + c
        q = v.astype(qdtype)
        out[..., s] = q
        c = v - q.astype(np.float32)
    return out


def _prep_host(hidden_state, encoder_output, W1, b1, w2):
    import ml_dtypes

    bf16 = ml_dtypes.bfloat16
    f8e4 = ml_dtypes.float8_e4m3
    f8e3 = ml_dtypes.float8_e3m4

    n_tiles = S // T
    n_pairs = n_tiles // 2
    encT = encoder_output.transpose(0, 2, 1)                 # [B, D, S] f32
    # score copy: e4m3, [b, pr, p, j, dc, s'] with d = dc*128 + p,
    # s = (2 pr + j) T + s'
    enc8 = np.ascontiguousarray(
        encT.reshape(B, DC, 128, n_pairs, 2, T)
        .transpose(0, 3, 2, 4, 1, 5).astype(f8e4)
    )
    # context copy: e3m4 diffused along s, [b, pr, p, j, c, d] with
    # s = (2 pr + j) T + 4 p + c
    encq = _diffuse_quant(encT, f8e3).astype(f8e3)           # [B, D, S]
    encq3 = np.ascontiguousarray(
        encq.transpose(0, 2, 1).reshape(B, n_pairs, 2, 128, 4, D)
        .transpose(0, 1, 3, 2, 4, 5)
    )
    w1eT = (W_SCALE * W1[:, :D].T).astype(f8e4)              # [d, k]
    w1et8 = np.ascontiguousarray(
        w1eT.reshape(DC, 128, KC, 128).transpose(2, 1, 0, 3)
    )
    w28 = np.ascontiguousarray(w2.reshape(KC, 128).astype(bf16))
    # r = W1hid @ hid + b1 on the host (tiny): [B, K]
    r_all = hidden_state @ W1[:, D:].T + b1
    in_maps = []
    for i in range(N_CORES):
        sl = slice(i * B_LOC, (i + 1) * B_LOC)
        in_maps.append({
            "enc8": np.ascontiguousarray(enc8[sl]),
            "encq3": np.ascontiguousarray(encq3[sl]),
            "w1et8": w1et8,
            "w28": w28,
            "r": np.ascontiguousarray(r_all[sl].astype(np.float32)),
        })
    return in_maps


def _ensure_ntff_hook():
    """Install the axon NTFF profile hook if the image lacks antenv.axon_hooks."""
    import sys
    import types

    try:
        import antenv.axon_hooks  # noqa: F401
        return
    except ImportError:
        pass
    import antenv

    mod = types.ModuleType("antenv.axon_hooks")
    state = {"hook": None}
    mod.set_axon_ntff_profile_hook = lambda h: state.__setitem__("hook", h)
    mod.get_axon_ntff_profile_hook = lambda: state["hook"]
    sys.modules["antenv.axon_hooks"] = mod
    antenv.axon_hooks = mod
    try:
        from trn_agent_boot.trn_boot import _ntff_profile_via_ctypes

        mod.set_axon_ntff_profile_hook(
            _ntff_profile_via_ctypes("/opt/axon/libaxon_pjrt.so")
        )
    except Exception:
        pass


def run(hidden_state, encoder_output, W1, b1, w2, trace=False):
    from concourse.bass_utils import run_bass_kernel_spmd

    if trace:
        _ensure_ntff_hook()

    nc = build_nc()
    nc.finalize()
    split_multi_waits(nc)
    in_maps = _prep_host(
        np.asarray(hidden_state, dtype=np.float32),
        np.asarray(encoder_output, dtype=np.float32),
        np.asarray(W1, dtype=np.float32),
        np.asarray(b1, dtype=np.float32),
        np.asarray(w2, dtype=np.float32),
    )
    res = run_bass_kernel_spmd(nc, in_maps, core_ids=list(range(N_CORES)), trace=trace)
    outs = []
    for i in range(N_CORES):
        ctx4 = res.results[i]["ctx4"]                    # [b_loc, 4, D]
        z = res.results[i]["z"]                          # [b_loc, 2, n_pairs]
        zb = z.reshape(B_LOC, -1).sum(axis=1)            # [b_loc]
        outs.append(ctx4.sum(axis=1) / zb[:, None])
    out = np.concatenate(outs, axis=0).astype(np.float32)
    return out, res


def kernel(**inputs):
    out, _ = run(**inputs)
    return out


# revision 32
# speedup vs baseline: 1.0580x; 1.0448x over previous
"""Bahdanau-attention pooling kernel for TRN2, data-parallel over 8 NeuronCores.

Reference computation (per batch b):
    h   = tanh(enc @ W1enc.T + hid @ W1hid.T + b1)    [S, K]   (K = D = 512)
    e   = h @ w2                                       [S]
    a   = softmax(e)                                   [S]
    ctx = a @ enc                                      [D]

Distribution: batch dim (32) sharded 4-per-core across 8 cores; replicated
weights, no collectives.

v4 design (per core, single pass over the encoder stream):
  - scores: enc ships as e4m3 [d, s] pair-tiles; W1enc pre-scaled x16 and
    quantized e4m3; h-matmuls run in DoubleRow fp8 mode (256-deep
    contraction per pass); the 1/16 rescale is folded into the tanh scale.
    tanh is j-pair fused (one ACT per kc chunk covers both tiles of a pair).
  - r = W1hid @ hid + b1 is computed on the HOST (tiny) and shipped as a
    [128, KC, b] column table -- no w1ht/hid/b1 on device.
  - e-matmuls (prev pair) run as FOUR concurrent 1-row chains in distinct
    PE column groups (q0/q32 = kc 0-1 halves, q64/q96 = kc 2-3 halves),
    packed into the kc0/kc1 h-windows; DVE adds the two halves. This halves
    the PE wall-time of the e stage and finishes it early enough that the
    exp -> pT chain never stalls the ctx matmuls.
  - exp runs with accum_out: z falls out of the activation for free (per
    batch it lands in a [33, n_pairs] slot table; rows 0/32 are j=0/1).
    Softmax normalization happens on the host: the kernel returns raw
    4-row ctx partials and the z table.
  - context: runs on the PE. enc also ships as e3m4 [s, d] pair-tiles
    (error-diffusion rounded along s so quantization noise cancels in the
    softmax average), with s interleaved as s = 4p + c so a plain
    [1,512]->[128,4] DMA produces the p-column tiles. Each tile adds 4
    rank-1 matmuls column-tiled to PSUM partitions {0,32,64,96} -- the four
    run fully concurrently on distinct col groups -- accumulated across the
    whole batch in one PSUM bank.
  - pT scatter DMAs ride the (otherwise idle) GpSimd queue; enc streaming
    rides Sync; the Scalar engine does activations only.
"""

import numpy as np

B, S, D = 32, 4096, 512
N_CORES = 8
B_LOC = B // N_CORES
T = 512          # s-tile size
KC = D // 128    # 4 k-chunks
DC = D // 128    # 4 d-chunks
W_SCALE = 16.0   # host pre-scale on W1enc before e4m3 quantization


def build_nc(b_loc=B_LOC, s_len=S, t=T):
    import concourse.bass as bass
    import concourse.mybir as mybir
    import concourse.tile as tile

    fp32 = mybir.dt.float32
    bf16 = mybir.dt.bfloat16
    f8e4 = mybir.dt.float8e4
    f8e3 = mybir.dt.float8e3
    AF = mybir.ActivationFunctionType
    Alu = mybir.AluOpType
    DR = mybir.MatmulPerfMode.DoubleRow

    nc = bass.Bass()

    n_tiles = s_len // t
    n_pairs_b = n_tiles // 2

    enc8_ext = nc.declare_dram_parameter(
        "enc8", [b_loc, n_pairs_b, 128, 2, DC, t], f8e4, isOutput=False)
    encq3_ext = nc.declare_dram_parameter(
        "encq3", [b_loc, n_pairs_b, 128, 2, 4, D], f8e3, isOutput=False)
    w1et8_ext = nc.declare_dram_parameter(
        "w1et8", [KC, 128, DC, 128], f8e4, isOutput=False)
    w28_ext = nc.declare_dram_parameter("w28", [KC, 128], bf16, isOutput=False)
    r_ext = nc.declare_dram_parameter("r", [b_loc, D], fp32, isOutput=False)
    ctx4_ext = nc.declare_dram_parameter("ctx4", [b_loc, 4, D], fp32, isOutput=True)
    z_ext = nc.declare_dram_parameter(
        "z", [b_loc, 2, n_pairs_b], fp32, isOutput=True)

    with tile.TileContext(nc) as tc:
        with (
            tc.tile_pool(name="singles", bufs=1) as singles,
            tc.tile_pool(name="enc8_pool", bufs=3) as enc8_pool,
            tc.tile_pool(name="enc3_pool", bufs=7) as enc3_pool,
            tc.tile_pool(name="h8_pool", bufs=3) as h8_pool,
            tc.tile_pool(name="p_pool", bufs=5) as p_pool,
            tc.tile_pool(name="pt_pool", bufs=6) as pt_pool,
            tc.tile_pool(name="esb_pool", bufs=5) as esb_pool,
            tc.tile_pool(name="ctxsb_pool", bufs=2) as ctxsb_pool,
            tc.tile_pool(name="z_pool", bufs=2) as z_pool,
            tc.tile_pool(name="ps_h", bufs=3, space=bass.MemorySpace.PSUM) as ps_h,
            tc.tile_pool(name="ps_e", bufs=1, space=bass.MemorySpace.PSUM) as ps_e,
            tc.tile_pool(name="ps_c", bufs=1, space=bass.MemorySpace.PSUM) as ps_c,
        ):
            # ---- persistent tiles; ordered so pair-0's dependencies land
            # first: w1et8[kc0] -> (enc8 pair 0 goes right after, see loop)
            w1et8_sb = singles.tile([128, KC, DC, 128], f8e4)
            w2_col = singles.tile([128, KC], bf16)
            r_sb = singles.tile([128, b_loc, KC], fp32)   # [p(k), b, k-chunk]
            nc.sync.dma_start(out=w1et8_sb[:, 0, :, :], in_=w1et8_ext[0])

            ctx_ps = ps_c.tile([128, t], fp32, tag="ctx")

            # ---- main loop: flat software-pipelined stream of tile pairs ---
            # Per pair P (b, it0) we emit, in order:
            #   DMAs(P) + deferred enc3 DMA(P-1)
            #   kc-loop: h-matmuls(P); at kc 0/1 the e-matmuls(P-1) run as 4
            #            concurrent col-group chains (kc-halves x j)
            #   DVE half-add + exp/z/pT(P-1)  (one ACT op, one transpose DMA)
            #   ctx-matmuls(P-2) (their pT landed during P-1 -- never stalls)
            # The pipeline runs across batch boundaries; each batch epilogue
            # is emitted right after that batch's last ctx-matmul flush.
            pairs = [(b, it0) for b in range(b_loc) for it0 in range(0, n_tiles, 2)]
            prev = None          # (h8, enc3t, b, it0) of pair P-1
            ctx_q = []           # pending ctx items: (pT, j, e3t, b, ti)
            zparts = {}          # per-batch z slot tables

            def emit_ctx(item):
                pT, j, e3t, cb, ti = item
                for c in range(4):
                    nc.tensor.matmul(
                        ctx_ps[32 * c:32 * c + 1, :],
                        pT[:, j, c:c + 1],
                        e3t[:, j, c, :],
                        start=(ti == 0),
                        stop=(ti == n_tiles - 1),
                        tile_position=(0, 32 * c),
                        skip_group_check=True,
                    )

            def emit_epilogue(eb):
                # raw 4-row ctx partials + z slots out; host sums/normalizes.
                # These ride Sync (not GpSimd) so the latency-critical pT
                # scatters never queue behind them.
                ctx_sb = ctxsb_pool.tile([97, t], fp32, tag="ctxsb")
                nc.vector.tensor_copy(out=ctx_sb, in_=ctx_ps[0:97, :])
                nc.sync.dma_start(
                    out=ctx4_ext[eb], in_=ctx_sb[0:97:32, :])
                zp = zparts.pop(eb)
                nc.sync.dma_start(out=z_ext[eb, 0], in_=zp[0:1, :])
                nc.sync.dma_start(out=z_ext[eb, 1], in_=zp[32:33, :])

            def emit_exp_block(pprev, split=False):
                # e halves add (DVE), exp (ACT), z row-sum (DVE), pT scatter
                # (GpSimd queue): all for pair P-1.  split=True runs exp per
                # j-row so the first pT scatter starts earlier (drain only).
                ph8, penc3, pb, pit0 = pprev
                ppi = pit0 // 2
                # DVE cannot read two PSUM operands in one op: stage the
                # upper half in SBUF first
                e_hi = esb_pool.tile([33, t], fp32, tag="ehi", name="e_hi")
                nc.vector.tensor_copy(out=e_hi, in_=e_ps[64:97, :])
                e_sb = esb_pool.tile([33, t], fp32, tag="esb")
                nc.vector.tensor_add(
                    out=e_sb, in0=e_ps[0:33, :], in1=e_hi)
                if ppi == 0:
                    zparts[pb] = z_pool.tile(
                        [33, n_pairs_b], fp32, tag="zp", name=f"zp{pb}")
                p_sb = p_pool.tile([33, t], bf16, tag="p")
                pT = pt_pool.tile([128, 2, 4], bf16, tag="pt")
                if split:
                    for j in range(2):
                        nc.scalar.activation(
                            out=p_sb[32 * j:32 * j + 1, :],
                            in_=e_sb[32 * j:32 * j + 1, :], func=AF.Exp)
                        nc.gpsimd.dma_start(
                            out=pT[:, j, :], in_=p_sb[32 * j:32 * j + 1, :])
                        nc.vector.tensor_reduce(
                            out=zparts[pb][32 * j:32 * j + 1, ppi:ppi + 1],
                            in_=p_sb[32 * j:32 * j + 1, :],
                            axis=mybir.AxisListType.X, op=Alu.add)
                else:
                    nc.scalar.activation(out=p_sb, in_=e_sb, func=AF.Exp)
                    for j in range(2):
                        nc.gpsimd.dma_start(
                            out=pT[:, j, :], in_=p_sb[32 * j:32 * j + 1, :])
                    nc.vector.tensor_reduce(
                        out=zparts[pb][:, ppi:ppi + 1], in_=p_sb,
                        axis=mybir.AxisListType.X, op=Alu.add)
                for j in range(2):
                    ctx_q.append((pT, j, penc3, pb, pit0 + j))

            pending_e3 = []
            pending_epi = []
            for pi, (b, it0) in enumerate(pairs):
                e8 = enc8_pool.tile([128, 2, DC, t], f8e4, tag="enc8")
                if pi == 0:
                    # split pair-0's load into dc-halves per j so the first
                    # kc iteration can start after 128 KB
                    for j in range(2):
                        for dh in range(2):
                            nc.sync.dma_start(
                                out=e8[:, j, 2 * dh:2 * dh + 2],
                                in_=enc8_ext[b, 0, :, j, 2 * dh:2 * dh + 2])
                else:
                    nc.sync.dma_start(out=e8, in_=enc8_ext[b, it0 // 2])
                e3 = enc3_pool.tile([128, 2, 4, D], f8e3, tag="enc3")
                pending_e3.append((e3, b, it0 // 2))
                if pi == 0:
                    # rest of the persistent loads ride the GpSimd ring (idle
                    # until pair 1's pT) so sync's queue holds only the
                    # critical enc stream
                    nc.scalar.dma_start(
                        out=r_sb, in_=r_ext.rearrange("b (c p) -> p b c", p=128))
                    nc.scalar.dma_start(out=w1et8_sb[:, 1, :, :], in_=w1et8_ext[1])
                    nc.scalar.dma_start(
                        out=w2_col, in_=w28_ext.rearrange("c p -> p c"))
                    for kc in range(2, KC):
                        nc.scalar.dma_start(
                            out=w1et8_sb[:, kc, :, :], in_=w1et8_ext[kc])
                while len(pending_e3) > 2:
                    pe3, pe3b, pe3pi = pending_e3.pop(0)
                    nc.sync.dma_start(out=pe3, in_=encq3_ext[pe3b, pe3pi])

                e_ps = ps_e.tile([128, t], fp32, tag="e", name="eps") if prev is not None else None

                # h8[p(k), kc, j, s] = tanh((1/16) h_ps + r)
                h8 = h8_pool.tile([128, KC, 2, t], bf16, tag="h8")
                for kc in range(KC):
                    h_ps = ps_h.tile([128, 2, t], fp32, tag="h")
                    for c2 in range(2):
                        for j in range(2):
                            nc.tensor.matmul(
                                h_ps[:, j, :],
                                w1et8_sb[:, kc, 2 * c2:2 * c2 + 2, :],
                                e8[:, j, 2 * c2:2 * c2 + 2, :],
                                start=(c2 == 0),
                                stop=(c2 == 1),
                                perf_mode=DR,
                                skip_group_check=True,
                            )
                    if prev is not None and kc < 2:
                        # e-matmuls of the previous pair: four 1-row chains
                        # (kc-half x j) on distinct PE column groups run
                        # concurrently; each chain is 2 steps (kc, kc+2)
                        for j in range(2):
                            nc.tensor.matmul(
                                e_ps[32 * j:32 * j + 1, :],
                                w2_col[:, kc:kc + 1],
                                prev[0][:, kc, j, :],
                                start=(kc == 0),
                                stop=(kc == 1),
                                tile_position=(0, 32 * j),
                                skip_group_check=True,
                            )
                            nc.tensor.matmul(
                                e_ps[64 + 32 * j:65 + 32 * j, :],
                                w2_col[:, kc + 2:kc + 3],
                                prev[0][:, kc + 2, j, :],
                                start=(kc == 0),
                                stop=(kc == 1),
                                tile_position=(0, 64 + 32 * j),
                                skip_group_check=True,
                            )
                    nc.scalar.activation(
                        out=h8[:, kc, :, :], in_=h_ps, func=AF.Tanh,
                        bias=r_sb[:, b, kc:kc + 1], scale=1.0 / W_SCALE,
                    )
                    if prev is not None and kc == 2:
                        emit_exp_block(prev)

                # batch epilogues are delayed one pair past their last ctx
                # quad so the PSUM->SBUF copy never head-of-line-blocks the
                # DVE FIFO (the e-half add behind it feeds exp -> pT -> ctx)
                while pending_epi:
                    emit_epilogue(pending_epi.pop(0))
                # flush ctx items 3-4 pairs old: the pT scatter DMA takes
                # ~3us on the small-descriptor queue -- worse while the
                # initial enc prefetch burst is in flight, so hold back
                # further early on
                thr = 8 if pi < 8 else 6
                while len(ctx_q) > thr:
                    item = ctx_q.pop(0)
                    emit_ctx(item)
                    if item[4] == n_tiles - 1:
                        # stop: the next item is the following batch's
                        # start=True quad, which must not overwrite ctx_ps
                        # before this batch's epilogue copy reads it
                        pending_epi.append(item[3])
                        break

                prev = (h8, e3, b, it0)

            # drain: e/exp/pT of the final pair, remaining ctx, last epilogue
            while pending_e3:
                pe3, pe3b, pe3pi = pending_e3.pop(0)
                nc.sync.dma_start(out=pe3, in_=encq3_ext[pe3b, pe3pi])
            # final pair uses plain 2-way chains (all four kc steps into rows
            # 0/32): no half-add needed, so exp can read PSUM directly -- the
            # drain's serial latency shrinks by the DVE copy+add.  The chains
            # go FIRST so the stale-quad backlog below overlaps the
            # exp -> scatter latency.
            e_ps = ps_e.tile([128, t], fp32, tag="e", name="eps_last")
            for kc in range(KC):
                for j in range(2):
                    nc.tensor.matmul(
                        e_ps[32 * j:32 * j + 1, :],
                        w2_col[:, kc:kc + 1],
                        prev[0][:, kc, j, :],
                        start=(kc == 0),
                        stop=(kc == KC - 1),
                        tile_position=(0, 32 * j),
                        skip_group_check=True,
                    )
            ph8, penc3, pb, pit0 = prev
            ppi = pit0 // 2
            p_sb = p_pool.tile([33, t], bf16, tag="p", name="p_last")
            pT = pt_pool.tile([128, 2, 4], bf16, tag="pt", name="pt_last")
            for j in range(2):
                nc.scalar.activation(
                    out=p_sb[32 * j:32 * j + 1, :],
                    in_=e_ps[32 * j:32 * j + 1, :], func=AF.Exp)
                nc.gpsimd.dma_start(
                    out=pT[:, j, :], in_=p_sb[32 * j:32 * j + 1, :])
                nc.vector.tensor_reduce(
                    out=zparts[pb][32 * j:32 * j + 1, ppi:ppi + 1],
                    in_=p_sb[32 * j:32 * j + 1, :],
                    axis=mybir.AxisListType.X, op=Alu.add)
            for j in range(2):
                ctx_q.append((pT, j, penc3, pb, pit0 + j))
            while pending_epi:
                emit_epilogue(pending_epi.pop(0))
            while ctx_q:
                item = ctx_q.pop(0)
                emit_ctx(item)
                if item[4] == n_tiles - 1:
                    emit_epilogue(item[3])

    return nc


# Instruction opcodes whose ISA structs tolerate multi-waits (or that the
# split must not touch). Everything else on this walrus build has a single
# sync-wait slot, so excess waits move onto preceding same-engine NoOps.
_NO_SPLIT = {"EventSemaphore", "Call", "UnconditionalBranch", "RegisterMove"}


def split_multi_waits(nc, limit=1):
    import concourse.mybir as mybir

    ctr = 0
    for fn in nc.m.functions:
        for blk in fn.blocks:
            new = []
            for inst in blk.instructions:
                si = inst.sync_info
                waits = list(si.on_wait) if si is not None and si.on_wait else []
                if inst.opcode not in _NO_SPLIT and len(waits) > limit:
                    extra, keep = waits[:-limit], waits[-limit:]
                    for w in extra:
                        ctr += 1
                        new.append(mybir.InstNoOp(
                            name=f"WSPLIT-{ctr}", engine=inst.engine,
                            sync_info=mybir.SyncInfo(on_wait=[w], on_update=[])))
                    inst.sync_info = mybir.SyncInfo(
                        on_wait=keep,
                        on_update=list(si.on_update) if si.on_update else [])
                new.append(inst)
            blk.instructions = new
    return ctr


def _diffuse_quant(x, qdtype):
    """Error-diffusion rounding of x (f32) to qdtype along the last axis:
    running quantization error is fed into the next element, so partial sums
    of the quantized stream track the exact partial sums within half an ULP.
    """
    out = np.empty(x.shape, dtype=qdtype)
    c = np.zeros(x.shape[:-1], dtype=np.float32)
    for s in range(x.shape[-1]):
        v = x[..., s] + c
        q = v.astype(qdtype)
        out[..., s] = q
        c = v - q.astype(np.float32)
    return out


def _prep_host(hidden_state, encoder_output, W1, b1, w2):
    import ml_dtypes

    bf16 = ml_dtypes.bfloat16
    f8e4 = ml_dtypes.float8_e4m3
    f8e3 = ml_dtypes.float8_e3m4

    n_tiles = S // T
    n_pairs = n_tiles // 2
    encT = encoder_output.transpose(0, 2, 1)                 # [B, D, S] f32
    # score copy: e4m3, [b, pr, p, j, dc, s'] with d = dc*128 + p,
    # s = (2 pr + j) T + s'
    enc8 = np.ascontiguousarray(
        encT.reshape(B, DC, 128, n_pairs, 2, T)
        .transpose(0, 3, 2, 4, 1, 5).astype(f8e4)
    )
    # context copy: e3m4 diffused along s, [b, pr, p, j, c, d] with
    # s = (2 pr + j) T + 4 p + c
    encq = _diffuse_quant(encT, f8e3).astype(f8e3)           # [B, D, S]
    encq3 = np.ascontiguousarray(
        encq.transpose(0, 2, 1).reshape(B, n_pairs, 2, 128, 4, D)
        .transpose(0, 1, 3, 2, 4, 5)
    )
    w1eT = (W_SCALE * W1[:, :D].T).astype(f8e4)              # [d, k]
    w1et8 = np.ascontiguousarray(
        w1eT.reshape(DC, 128, KC, 128).transpose(2, 1, 0, 3)
    )
    w28 = np.ascontiguousarray(w2.reshape(KC, 128).astype(bf16))
    # r = W1hid @ hid + b1 on the host (tiny): [B, K]
    r_all = hidden_state @ W1[:, D:].T + b1
    in_maps = []
    for i in range(N_CORES):
        sl = slice(i * B_LOC, (i + 1) * B_LOC)
        in_maps.append({
            "enc8": np.ascontiguousarray(enc8[sl]),
            "encq3": np.ascontiguousarray(encq3[sl]),
            "w1et8": w1et8,
            "w28": w28,
            "r": np.ascontiguousarray(r_all[sl].astype(np.float32)),
        })
    return in_maps


def _ensure_ntff_hook():
    """Install the axon NTFF profile hook if the image lacks antenv.axon_hooks."""
    import sys
    import types

    try:
        import antenv.axon_hooks  # noqa: F401
        return
    except ImportError:
        pass
    import antenv

    mod = types.ModuleType("antenv.axon_hooks")
    state = {"hook": None}
    mod.set_axon_ntff_profile_hook = lambda h: state.__setitem__("hook", h)
    mod.get_axon_ntff_profile_hook = lambda: state["hook"]
    sys.modules["antenv.axon_hooks"] = mod
    antenv.axon_hooks = mod
    try:
        from trn_agent_boot.trn_boot import _ntff_profile_via_ctypes

        mod.set_axon_ntff_profile_hook(
            _ntff_profile_via_ctypes("/opt/axon/libaxon_pjrt.so")
        )
    except Exception:
        pass


def run(hidden_state, encoder_output, W1, b1, w2, trace=False):
    from concourse.bass_utils import run_bass_kernel_spmd

    if trace:
        _ensure_ntff_hook()

    nc = build_nc()
    nc.finalize()
    split_multi_waits(nc)
    in_maps = _prep_host(
        np.asarray(hidden_state, dtype=np.float32),
        np.asarray(encoder_output, dtype=np.float32),
        np.asarray(W1, dtype=np.float32),
        np.asarray(b1, dtype=np.float32),
        np.asarray(w2, dtype=np.float32),
    )
    res = run_bass_kernel_spmd(nc, in_maps, core_ids=list(range(N_CORES)), trace=trace)
    outs = []
    for i in range(N_CORES):
        ctx4 = res.results[i]["ctx4"]                    # [b_loc, 4, D]
        z = res.results[i]["z"]                          # [b_loc, 2, n_pairs]
        zb = z.reshape(B_LOC, -1).sum(axis=1)            # [b_loc]
        outs.append(ctx4.sum(axis=1) / zb[:, None])
    out = np.concatenate(outs, axis=0).astype(np.float32)
    return out, res


def kernel(**inputs):
    out, _ = run(**inputs)
    return out
